# revision 6
# baseline (speedup 1.0000x reference)
"""Trainium2 Bass kernel for nn_Block_32762010534337 (dense transformer block).

Strategy: stride-4 interleaved sequence parallel over 8 cores. Core c owns
tokens {4i + g} (g = c%4) of batch c//4 -- every core then has an IDENTICAL
causal attention program (q-tile qt needs exactly 8*(qt+1) key tiles); all
causality lives in per-core mask/index data. K/V are projected for the core's
own 512 tokens, AllGathered within the 4-core batch group (overlapped with Q
projection), and re-tiled into global key tiles via strided APs (K) and
partition-scatter DMAs (V). Scores are computed per 256-query tile into 4-tile
PSUM slabs, exp'd in one ScalarE ACTIVATE per slab, softmax denominators
accumulate on the PE (separate bank), and 1/x is computed as exp(-ln(x)) on
rows only. Activations stay feature-major end-to-end: the MLP proj output is
produced feature-major (no PE transposes) and the host transposes/scatters the
final [C, 512] tiles for free.
"""
import sys
import os

if "/opt/trn_rl_repo" not in sys.path:
    sys.path.insert(0, "/opt/trn_rl_repo")

import numpy as np

B, T, C = 2, 2048, 2048
NH, NKV, HD = 16, 4, 128
DFF = 4 * C
TQ = 512          # tokens per core (stride-4 interleaved)
CH = 256          # query tile
NT = C // 128     # 16 feature tiles
NF = DFF // 128   # 64 ff tiles
EPS = 1.1920929e-07
NCORES = 8

_CACHE = None


def _build():
    import concourse.bass as bass
    import concourse.tile as tile
    from concourse import mybir, bacc

    dt = mybir.dt
    f32, bf16 = dt.float32, dt.bfloat16
    Alu = mybir.AluOpType
    Act = mybir.ActivationFunctionType

    nc = bacc.Bacc("TRN2", target_bir_lowering=False, debug=False, num_devices=NCORES)

    for val in (EPS, HD * EPS):
        tns = nc.alloc_sbuf_tensor(f"const-f32-{val}", [128, 1], f32)
        nc.gpsimd.memset(tns.ap(), val)
        nc.const_aps.aps[(f32, val)] = tns.ap()
    nc.all_engine_barrier()

    xT = nc.declare_dram_parameter("xT", [C, TQ], bf16, isOutput=False)
    csc = nc.declare_dram_parameter("csc", [128, TQ], bf16, isOutput=False)
    css = nc.declare_dram_parameter("css", [128, TQ], bf16, isOutput=False)
    mask = nc.declare_dram_parameter("mask", [128, 8 * CH], bf16, isOutput=False)
    # weights host-pretiled (see _prep_weights)
    wq = nc.declare_dram_parameter("wq", [128, 4 * NT * TQ], bf16, isOutput=False)
    wk = nc.declare_dram_parameter("wk", [128, NT * TQ], bf16, isOutput=False)
    wv = nc.declare_dram_parameter("wv", [128, NT * TQ], bf16, isOutput=False)
    wo = nc.declare_dram_parameter("wo", [128, 4 * NT * TQ], bf16, isOutput=False)
    wfc = nc.declare_dram_parameter("wfc", [128, 16 * NT * TQ], bf16, isOutput=False)
    wproj = nc.declare_dram_parameter("wproj", [128, NT * NF * 128], bf16,
                                      isOutput=False)
    out_fm = nc.declare_dram_parameter("out", [C, TQ], f32, isOutput=True)

    ck_in = nc.dram_tensor("ck_in", [512, TQ], bf16)
    ck_out = nc.dram_tensor("ck_out", [2048, TQ], bf16)
    cv_in = nc.dram_tensor("cv_in", [512, TQ], bf16)
    cv_out = nc.dram_tensor("cv_out", [2048, TQ], bf16)

    with tile.TileContext(nc, num_cores=NCORES) as tc:
        with (
            tc.tile_pool(name="const", bufs=1) as constp,
            tc.tile_pool(name="persist", bufs=1) as pp,
            tc.tile_pool(name="work", bufs=3) as wpool,
            tc.tile_pool(name="wstream", bufs=3) as wsp,
        ):
            ones = constp.tile([128, 1], bf16, tag="ones")
            nc.gpsimd.memset(ones, 1.0)

            # x_mid^T lives across attention + MLP
            xmT = pp.tile([128, NT, TQ], f32, tag="xmT")

            def norm_row(ssq_ps, scale, bias, n, nb=128):
                """[1,n] psum sum-of-squares -> [nb,n] f32 bcast of
                (scale*x+bias)^(-1/2), via exp(-0.5*ln(.))."""
                ln = wpool.tile([1, n], f32, tag="srow", bufs=4, name="lnrow")
                nc.scalar.activation(ln[:], ssq_ps[:], Act.Ln, bias=bias,
                                     scale=scale)
                rs = wpool.tile([1, n], f32, tag="srow", bufs=4, name="rsrow")
                nc.scalar.activation(rs[:], ln[:], Act.Exp, scale=-0.5)
                sb = wpool.tile([nb, n], f32, tag="sbcast", bufs=3)
                nc.gpsimd.partition_broadcast(sb[:], rs[:])
                return sb

            def wslab2(param, base, width, n_i, name):
                """Stream [128, n_i, width] weights as two 8KB half-slabs."""
                half = n_i // 2
                tiles = []
                for hh in range(2):
                    ts = wsp.tile([128, half, width], bf16, tag="wslab",
                                  bufs=3, name=f"{name}_{hh}")
                    o = base + half * width * hh
                    nc.sync.dma_start(
                        ts[:],
                        param[:, o:o + half * width].rearrange(
                            "p (g t) -> p g t", t=width))
                    tiles.append(ts)

                def get(i, c0=None, c1=None):
                    t, j = tiles[i // half], i % half
                    return t[:, j] if c0 is None else t[:, j, c0:c1]
                return get

            with tc.tile_pool(name="main", bufs=1) as mp:
                csc_sb = mp.tile([128, TQ], bf16, tag="csc")
                nc.sync.dma_start(csc_sb[:], csc[:])
                css_sb = mp.tile([128, TQ], bf16, tag="css")
                nc.sync.dma_start(css_sb[:], css[:])
                mask_sb = mp.tile([128, 8, CH], bf16, tag="mask_sb")
                nc.sync.dma_start(mask_sb[:],
                                  mask.rearrange("p (d q) -> p d q", q=CH))

                hT = mp.tile([128, NT, TQ], bf16, tag="hT")
                qs_sb = mp.tile([128, NH, TQ], bf16, tag="qs_sb")
                k_sb = mp.tile([128, 4, NKV, TQ], bf16, tag="k_sb")
                v_sb = mp.tile([128, 4, 4, NKV * HD], bf16, tag="v_sb")
                yT = mp.tile([128, NH, TQ], bf16, tag="yT")

                def load_xT(i):
                    xt = mp.tile([128, TQ], bf16, tag="xin", bufs=2,
                                 name="xin")
                    nc.sync.dma_start(xt[:],
                                      xT[128 * i:128 * (i + 1), :])
                    return xt

                def rope(ps, pool):
                    """psum [128,TQ] f32 -> rope'd bf16 sbuf tile."""
                    raw = pool.tile([128, TQ], bf16, tag="rraw", bufs=3,
                                    name="rraw")
                    nc.scalar.copy(raw[:], ps[:])
                    sw = pool.tile([128, TQ], bf16, tag="rsw", bufs=2,
                                   name="rsw")
                    nc.sync.dma_start(sw[0:64, :], raw[64:128, :])
                    nc.sync.dma_start(sw[64:128, :], raw[0:64, :])
                    rr = pool.tile([128, TQ], bf16, tag="rr", bufs=4,
                                   name="rr")
                    nc.vector.tensor_tensor(rr[:], raw[:], csc_sb[:], Alu.mult)
                    t2 = pool.tile([128, TQ], bf16, tag="rt2", bufs=2,
                                   name="rt2")
                    nc.vector.tensor_tensor(t2[:], sw[:], css_sb[:], Alu.mult)
                    nc.vector.tensor_tensor(rr[:], rr[:], t2[:], Alu.add)
                    return rr

                def sumsq(rr, pool):
                    sq = pool.tile([128, TQ], bf16, tag="rsq", bufs=4,
                                   name="rsq")
                    nc.vector.tensor_tensor(sq[:], rr[:], rr[:], Alu.mult)
                    return sq

                with tc.tile_pool(name="psA", bufs=1, space="PSUM") as psA:
                    # ---- P0: pre-attention rmsnorm (feature-major) ----
                    ssq_ps = psA.tile([1, TQ], f32, tag="row", bufs=3)
                    for i in range(NT):
                        xt = load_xT(i)
                        xsq = wpool.tile([128, TQ], bf16, tag="xsq", bufs=6)
                        nc.vector.tensor_tensor(xsq[:], xt[:], xt[:],
                                                Alu.mult)
                        nc.tensor.matmul(ssq_ps[:], lhsT=ones[:], rhs=xsq[:],
                                         start=(i == 0), stop=(i == NT - 1))
                    s1b = norm_row(ssq_ps, 1.0 / C, EPS, TQ)
                    for i in range(NT):
                        xt = load_xT(i)
                        nc.vector.tensor_tensor(hT[:, i], xt[:], s1b[:],
                                                Alu.mult)

                    # ---- K heads: project + rope + k-norm -> ck_in ----
                    kps = [psA.tile([128, TQ], f32, tag="qkv", bufs=4,
                                    name=f"kps_{_k}") for _k in range(4)]
                    wkf = wslab2(wk, 0, TQ, NT, "wk")
                    for k in range(4):
                        for i in range(NT):
                            nc.tensor.matmul(kps[k][:],
                                             lhsT=wkf(i, 128 * k, 128 * (k + 1)),
                                             rhs=hT[:, i],
                                             start=(i == 0), stop=(i == NT - 1))
                    for kh in range(4):
                        rr = rope(kps[kh], mp)
                        sq = sumsq(rr, mp)
                        sps = psA.tile([1, TQ], f32, tag="row", bufs=3)
                        nc.tensor.matmul(sps[:], lhsT=ones[:], rhs=sq[:],
                                         start=True, stop=True)
                        sb = norm_row(sps, 1.0 / HD, EPS, TQ)
                        kt = mp.tile([128, TQ], bf16, tag="ktile", bufs=3,
                                     name="kt")
                        nc.vector.tensor_tensor(kt[:], rr[:], sb[:], Alu.mult)
                        nc.sync.dma_start(ck_in[128 * kh:128 * (kh + 1), :],
                                          kt[:])

                    nc.gpsimd.collective_compute(
                        "AllGather", Alu.bypass,
                        replica_groups=[[0, 1, 2, 3], [4, 5, 6, 7]],
                        ins=[ck_in[:]], outs=[ck_out[:]])

                    # ---- V heads: project directly token-major -> cv_in ----
                    wvf = wslab2(wv, 0, TQ, NT, "wv")
                    for t in range(4):
                        vps = psA.tile([128, TQ], f32, tag="qkv", bufs=4,
                                       name=f"vps_{t}")
                        for i in range(NT):
                            nc.tensor.matmul(vps[:],
                                             lhsT=hT[:, i, 128 * t:128 * (t + 1)],
                                             rhs=wvf(i),
                                             start=(i == 0), stop=(i == NT - 1))
                        vb = wpool.tile([128, TQ], bf16, tag="vb", bufs=2)
                        nc.vector.tensor_copy(out=vb[:], in_=vps[:])
                        nc.sync.dma_start(cv_in[128 * t:128 * (t + 1), :],
                                          vb[:])
                    nc.gpsimd.collective_compute(
                        "AllGather", Alu.bypass,
                        replica_groups=[[0, 1, 2, 3], [4, 5, 6, 7]],
                        ins=[cv_in[:]], outs=[cv_out[:]])

                    # ---- Q heads: project + rope + deferred q-norm ----
                    pending = None

                    def finish_q(pend):
                        hg, rrs, sqs = pend
                        for k in range(4):
                            h = 4 * hg + k
                            sps = psA.tile([1, TQ], f32, tag="row", bufs=3)
                            nc.tensor.matmul(sps[:], lhsT=ones[:],
                                             rhs=sqs[k][:],
                                             start=True, stop=True)
                            sb = norm_row(sps, 1.0, HD * EPS, TQ)
                            nc.vector.tensor_tensor(qs_sb[:, h], rrs[k][:],
                                                    sb[:], Alu.mult)

                    for hg in range(4):
                        qps = [psA.tile([128, TQ], f32, tag="qkv", bufs=4,
                                        name=f"qps{hg}_{_k}") for _k in range(4)]
                        wqf = wslab2(wq, NT * TQ * hg, TQ, NT, f"wq{hg}")
                        rrs = []
                        sqs = []
                        for k in range(4):
                            for i in range(NT):
                                nc.tensor.matmul(qps[k][:],
                                                 lhsT=wqf(i, 128 * k, 128 * (k + 1)),
                                                 rhs=hT[:, i],
                                                 start=(i == 0),
                                                 stop=(i == NT - 1))
                            rrs.append(rope(qps[k], mp))
                            sqs.append(sumsq(rrs[k], mp))
                        if pending is not None:
                            finish_q(pending)
                        pending = (hg, rrs, sqs)
                    finish_q(pending)

                # ---- load gathered K/V (key tile m = rank m%4, its
                # column block m//4; contiguous in both buffers) ----
                for gp in range(4):
                    nc.sync.dma_start(
                        k_sb[:, gp],
                        ck_out[512 * gp:512 * (gp + 1), :].rearrange(
                            "(kh p) t -> p kh t", p=128))
                    nc.sync.dma_start(
                        v_sb[:, gp],
                        cv_out[512 * gp:512 * (gp + 1), :].rearrange(
                            "(cb p) f -> p cb f", p=128))

                # ---- attention ----
                with tc.tile_pool(name="psB", bufs=1, space="PSUM") as psB:
                    for kh in range(NKV):
                        for j in range(4):
                            h = 4 * kh + j
                            for qt in range(2):
                                nk = 8 * (qt + 1)
                                dlo = 8 * qt
                                y_ps = psB.tile([128, CH], f32, tag="y",
                                                bufs=2)
                                den_ps = psB.tile([1, CH], f32, tag="den",
                                                  bufs=2)
                                for grp in range(nk // 4):
                                    sc = psB.tile([128, 4, CH], f32, tag="sc",
                                                  bufs=2)
                                    p_sb = mp.tile([128, 4, CH], bf16,
                                                   tag="p_sb", bufs=2)
                                    for mi in range(4):
                                        m = 4 * grp + mi
                                        gp, cb = m % 4, m // 4
                                        nc.tensor.matmul(
                                            sc[:, mi],
                                            lhsT=k_sb[:, gp, kh,
                                                      128 * cb:128 * (cb + 1)],
                                            rhs=qs_sb[:, h,
                                                      CH * qt:CH * (qt + 1)],
                                            start=True, stop=True)
                                    nc.scalar.activation(p_sb[:], sc[:],
                                                         Act.Exp)
                                    for mi in range(4):
                                        m = 4 * grp + mi
                                        gp, cb = m % 4, m // 4
                                        if m >= dlo:
                                            nc.vector.tensor_tensor(
                                                p_sb[:, mi], p_sb[:, mi],
                                                mask_sb[:, m - dlo], Alu.mult)
                                        nc.tensor.matmul(
                                            y_ps[:],
                                            lhsT=v_sb[:, gp, cb,
                                                      128 * kh:128 * (kh + 1)],
                                            rhs=p_sb[:, mi],
                                            start=(m == 0), stop=(m == nk - 1))
                                        nc.tensor.matmul(
                                            den_ps[:], lhsT=ones[:],
                                            rhs=p_sb[:, mi],
                                            start=(m == 0), stop=(m == nk - 1))
                                # y /= den  via  exp(-ln(den))
                                lnr = wpool.tile([1, CH], f32, tag="srow",
                                                 bufs=4, name="lnden")
                                nc.scalar.activation(lnr[:], den_ps[:], Act.Ln)
                                rcp = wpool.tile([1, CH], f32, tag="srow",
                                                 bufs=4, name="rcpden")
                                nc.scalar.activation(rcp[:], lnr[:], Act.Exp,
                                                     scale=-1.0)
                                db = wpool.tile([128, CH], f32, tag="dbcast",
                                                bufs=2)
                                nc.gpsimd.partition_broadcast(db[:], rcp[:])
                                nc.vector.tensor_tensor(
                                    yT[:, h, CH * qt:CH * (qt + 1)],
                                    y_ps[:], db[:], Alu.mult)

                # ---- wo projection + residual (feature-major xmT) ----
                with tc.tile_pool(name="psC", bufs=1, space="PSUM") as psC:
                    for n4 in range(4):
                        wof = wslab2(wo, NT * TQ * n4, TQ, NT, f"wo{n4}")
                        for k in range(4):
                            att_ps = psC.tile([128, TQ], f32, tag="att",
                                              bufs=4)
                            for hh in range(NH):
                                nc.tensor.matmul(
                                    att_ps[:],
                                    lhsT=wof(hh, 128 * k, 128 * (k + 1)),
                                    rhs=yT[:, hh, :],
                                    start=(hh == 0), stop=(hh == NH - 1))
                            n = 4 * n4 + k
                            xt = load_xT(n)
                            nc.vector.tensor_tensor(xmT[:, n], att_ps[:],
                                                    xt[:], Alu.add)
            # main pool closed (frees attention SBUF)

            # ---- MLP ----
            with tc.tile_pool(name="mlp", bufs=1) as mlpp:
                h2T = mlpp.tile([128, NT, TQ], bf16, tag="h2T")
                a_sb = mlpp.tile([128, NF, TQ], bf16, tag="a_sb")

                with tc.tile_pool(name="psC2", bufs=1, space="PSUM") as psC2:
                    ssq2 = psC2.tile([1, TQ], f32, tag="row", bufs=2)
                    for i in range(NT):
                        xsq = wpool.tile([128, TQ], bf16, tag="xsq", bufs=6)
                        nc.vector.tensor_tensor(xsq[:], xmT[:, i], xmT[:, i],
                                                Alu.mult)
                        nc.tensor.matmul(ssq2[:], lhsT=ones[:], rhs=xsq[:],
                                         start=(i == 0), stop=(i == NT - 1))
                    s2b = norm_row(ssq2, 1.0 / C, EPS, TQ)
                    for i in range(NT):
                        nc.vector.tensor_tensor(h2T[:, i], xmT[:, i], s2b[:],
                                                Alu.mult)

                # fc + relu^2 (feature-major a)
                with tc.tile_pool(name="psD", bufs=1, space="PSUM") as psD:
                    for jc in range(16):
                        wfcf = wslab2(wfc, NT * TQ * jc, TQ, NT, f"wfc{jc}")
                        for jf in range(4):
                            f_ps = psD.tile([128, TQ], f32, tag="f", bufs=6)
                            for i in range(NT):
                                nc.tensor.matmul(
                                    f_ps[:],
                                    lhsT=wfcf(i, 128 * jf, 128 * (jf + 1)),
                                    rhs=h2T[:, i],
                                    start=(i == 0), stop=(i == NT - 1))
                            f = 4 * jc + jf
                            r_bf = wpool.tile([128, TQ], bf16, tag="r_bf")
                            nc.scalar.activation(r_bf[:], f_ps[:], Act.Relu)
                            nc.vector.tensor_tensor(a_sb[:, f], r_bf[:],
                                                    r_bf[:], Alu.mult)

                # proj: weight-stationary, feature-major output + residual
                with tc.tile_pool(name="psE", bufs=1, space="PSUM") as psE:
                    for n in range(16):
                        wpf = wslab2(wproj, NF * 128 * n, 128, NF, f"wp{n}")
                        o_ps = psE.tile([128, TQ], f32, tag="o", bufs=3)
                        for f in range(NF):
                            nc.tensor.matmul(o_ps[:], lhsT=wpf(f),
                                             rhs=a_sb[:, f, :],
                                             start=(f == 0),
                                             stop=(f == NF - 1))
                        ov = wpool.tile([128, TQ], f32, tag="ov", bufs=3)
                        nc.vector.tensor_tensor(ov[:], o_ps[:], xmT[:, n],
                                                Alu.add)
                        nc.sync.dma_start(out_fm[128 * n:128 * (n + 1), :],
                                          ov[:])

    nc.compile()
    return nc


def _prep_weights(wq, wk, wv, wo, w_fc, w_proj):
    import ml_dtypes
    bf = ml_dtypes.bfloat16

    def tile_w(w, chunk):
        # [R, F] -> [128, (F//chunk) * (R//128) * chunk]
        R, F = w.shape
        t = w.reshape(R // 128, 128, F // chunk, chunk)
        t = t.transpose(1, 2, 0, 3)
        return np.ascontiguousarray(t.reshape(128, -1)).astype(bf)

    return {
        "wq": tile_w(np.asarray(wq, np.float32), TQ),
        "wk": tile_w(np.asarray(wk, np.float32), NKV * HD),
        "wv": tile_w(np.asarray(wv, np.float32), NKV * HD),
        "wo": tile_w(np.asarray(wo, np.float32), TQ),
        "wfc": tile_w(np.asarray(w_fc, np.float32), TQ),
        "wproj": tile_w(np.asarray(w_proj, np.float32), 128),
    }


def _make_in_maps(x, cos, sin, weights_b):
    import ml_dtypes
    bf = ml_dtypes.bfloat16
    cosT = cos[0, :, 0, :].T  # [64, T]
    sinT = sin[0, :, 0, :].T
    kk = np.arange(128)
    qq = np.arange(CH)
    dd = np.arange(8)                           # band tile: d = dd//4, gp = dd%4
    in_maps = []
    for c in range(NCORES):
        b, g = divmod(c, 4)
        idx = 4 * np.arange(TQ) + g             # own token positions
        # key pos = 512*(2qt+d) + 4k + gp ; query pos = 1024qt + 4q + g
        off = 128 * (dd // 4) + (dd % 4 > g)
        msk = (qq[None, None, :] - kk[:, None, None]
               >= off[None, :, None]).astype(np.float32)
        m = {
            "xT": np.ascontiguousarray(x[b, idx, :].T).astype(bf),
            "csc": np.ascontiguousarray(
                np.concatenate([cosT[:, idx], cosT[:, idx]],
                               axis=0)).astype(bf),
            "css": np.ascontiguousarray(
                np.concatenate([sinT[:, idx], -sinT[:, idx]],
                               axis=0)).astype(bf),
            "mask": np.ascontiguousarray(msk.reshape(128, 8 * CH)).astype(bf),
        }
        m.update(weights_b)
        in_maps.append(m)
    return in_maps


def kernel(x, cos, sin, wq, wk, wv, wo, w_fc, w_proj):
    global _CACHE
    from concourse.bass_utils import run_bass_kernel_spmd

    x = np.asarray(x, np.float32)
    cos = np.asarray(cos, np.float32)
    sin = np.asarray(sin, np.float32)
    weights_b = _prep_weights(wq, wk, wv, wo, w_fc, w_proj)

    if _CACHE is None:
        _CACHE = _build()
    nc = _CACHE

    in_maps = _make_in_maps(x, cos, sin, weights_b)
    res = run_bass_kernel_spmd(nc, in_maps, list(range(NCORES)))
    out = np.empty((B, T, C), np.float32)
    for c in range(NCORES):
        b, g = divmod(c, 4)
        idx = 4 * np.arange(TQ) + g
        out[b, idx, :] = res.results[c]["out"].T
    return out
